# revision 20
# baseline (speedup 1.0000x reference)
"""Trainium2 Bass/Tile kernel: batched dot-product attention with length masking.

Problem: queries/keys/values [32, 1024, 128] f32, valid_length [32] int64.
  out = softmax(mask(Q K^T / sqrt(128))) @ V

Strategy:
  - Data-parallel: 32 batches sharded 4-per-core across 8 NeuronCores (SPMD,
    identical program, per-core input maps).
  - Host prep per batch (layout only; every tensor is a single fully
    contiguous DMA so descriptors aggregate into large packets):
      qT      [128=D, 1024] f32->fp16  (contraction dim on partitions)
      k{b}    [128=D, KC]   fp16       (K^T trimmed to the live k-blocks)
      v{b}    [128, KB*128] fp16       (V partition-major per k-block)
      fb{b}   [128, KB]     f32        exp-bias: 0 for valid k, -1e4 masked
  - Device per batch (matmul passes stream 512-row moving operands so the
    PE keeps its stationary loaded across 1024 rows):
      S^T[k, q] = (K^T_kb).T @ Q^T           PE
      P^T_kb    = exp(S^T*scale + fb[:,kb])  ScalarE PSUM->SBUF fp16.
                  The per-partition bias is -1e4 on masked k rows, so exp
                  underflows to exactly 0 there: masking costs nothing and
                  no separate mask matmul or V-zeroing is needed.
      pacc      = sum_kb P^T_kb              DVE adds (cheap, off PE)
      den[1,q]  = ones.T @ pacc              PE, only 2x512 rows per batch
                                             (vs KB*2x512 for a full
                                             mask-stationary den pass)
      O^T[v,q]  = sum_kb V_kb @ P^T_kb       PE, V stationary
    The last batch skips pacc and accumulates den over the P tiles directly
    on the PE (KB is smallest there after the sort) so the tail has no
    DVE dependency. Host does out = O^T.T / den in f32.
    No rowmax subtraction needed: scores ~ N(0,1), |S*scale| <~ 6.
  - DMA issues avoid GpSimd entirely: its DGE ring is software-managed and
    costs ~3us in the end-of-kernel drain (sync/scalar/vector rings are HW).
    k0 goes on scalar + q0 on sync so both batch-0 S operands issue in
    parallel at t=0; v's on vector; everything else on sync, with batch b+1
    loads emitted before den_pv(b-1) so output DMAs never delay loads.
  - A dummy 1-column exp at kernel start pulls the ~1.3us ACT_TABLE_LOAD
    into the initial DMA shadow (the compiler inserts it before the first
    Exp on the scalar engine).
  - Length specialization: batches sorted by valid_length desc, assigned
    round-robin so slot j is similar across cores; program compiled per
    kb_counts skips fully-masked k-blocks.
"""

import os

import numpy as np
import ml_dtypes

import concourse.tile as tile
from concourse import bacc, mybir
from concourse.bass_utils import run_bass_kernel_spmd

B, Q, K, D = 32, 1024, 1024, 128
N_CORES = 8
BPC = B // N_CORES  # batches per core
KB_MAX = K // 128
QH = 512
SCALE = float(1.0 / np.sqrt(D))
MASK_BIAS = -10000.0  # exp(s*scale + MASK_BIAS) underflows to exactly 0

S_DTYPE = os.environ.get("ATTN_S_DTYPE", "fp16")  # fp16 | bf16 | f32r | f32
NO_SPECIALIZE = os.environ.get("ATTN_NO_SPECIALIZE", "0") == "1"
N_WARM = int(os.environ.get("ATTN_WARM", "7"))

LAST_RESULTS = None
_NC_CACHE: dict = {}


def _dtypes(sdt):
    """(qk_dt for Q/K/S-matmul, ldt for P/V/ones)."""
    f32 = mybir.dt.float32
    qk = {"fp16": mybir.dt.float16, "bf16": mybir.dt.bfloat16,
          "f32r": mybir.dt.float32r, "f32": f32}[sdt]
    ldt = mybir.dt.float16 if sdt == "fp16" else mybir.dt.bfloat16
    return qk, ldt


def _body(tc, qT, kts, vts, fbs, outT, den, kb_counts, sdt):
    nc = tc.nc
    f32 = mybir.dt.float32
    AF = mybir.ActivationFunctionType
    qk_dt, ldt = _dtypes(sdt)

    with (
        tc.tile_pool(name="qk", bufs=3) as qk_pool,
        tc.tile_pool(name="v", bufs=3) as v_pool,
        tc.tile_pool(name="p", bufs=2) as p_pool,
        tc.tile_pool(name="pa", bufs=2) as pa_pool,
        tc.tile_pool(name="fb", bufs=3) as fb_pool,
        tc.tile_pool(name="eps", bufs=2) as e_pool,
        tc.tile_pool(name="const", bufs=1) as c_pool,
        tc.tile_pool(name="spsum", bufs=2, space="PSUM") as s_pool,
        tc.tile_pool(name="opsum", bufs=1, space="PSUM") as o_pool,
        tc.tile_pool(name="dpsum", bufs=1, space="PSUM") as d_pool,
    ):
        KBM = max(kb_counts)

        def load_batch(b):
            # per-tag tile shapes are constant (max KB) so the pool slot
            # size doesn't depend on allocation order; DMAs/compute slice
            KB = kb_counts[b]
            KC = KB * 128
            q_sb = qk_pool.tile([128, Q], qk_dt, tag="q", name=f"q_sb{b}")
            k_sb = qk_pool.tile([128, KBM * 128], qk_dt, tag="k",
                                name=f"k_sb{b}")
            v_sb = v_pool.tile([128, KBM * 128], ldt, tag="v",
                               name=f"v_sb{b}")
            fb_sb = fb_pool.tile([128, KBM], f32, tag="fb",
                                 name=f"fb_sb{b}")
            # two HW DGE rings (only gpsimd/SP/ACT can issue DMAs; gpsimd's
            # software ring costs ~3us in the final drain, so it issues
            # nothing). Slots are ordered smallest-KB first, so batch 0's k
            # is tiny: k0 rides sync while the full q0 rides scalar, and
            # both batch-0 S operands land ~3.5us (the fixed DGE latency)
            # after kernel entry. k1-k3 keep the scalar ring busy before its
            # exp stream starts.
            if b == 0:
                nc.sync.dma_start(out=k_sb[:, 0:KC], in_=kts[b][:])
                nc.scalar.dma_start(out=q_sb[:], in_=qT[b])
                nc.sync.dma_start(out=fb_sb[:, 0:KB], in_=fbs[b][:])
                nc.sync.dma_start(out=v_sb[:, 0:KC], in_=vts[b][:])
                return q_sb, k_sb, v_sb, fb_sb
            elif b == 1:
                # k1 is needed early: 2nd slot on the scalar ring (right
                # after q0). Completion is ~issue-end + 3.5us fixed DGE
                # latency, so queue position directly sets arrival time.
                nc.scalar.dma_start(out=k_sb[:, 0:KC], in_=kts[b][:])
                nc.sync.dma_start(out=q_sb[:], in_=qT[b])
            else:
                nc.scalar.dma_start(out=k_sb[:, 0:KC], in_=kts[b][:])
                nc.sync.dma_start(out=q_sb[:], in_=qT[b])
            nc.sync.dma_start(out=fb_sb[:, 0:KB], in_=fbs[b][:])
            nc.sync.dma_start(out=v_sb[:, 0:KC], in_=vts[b][:])
            return q_sb, k_sb, v_sb, fb_sb

        def s_exp_one(b, kb, q_sb, k_sb, fb_sb, p_all):
            s_ps = s_pool.tile([128, Q], f32, tag="s", name=f"s_ps{b}_{kb}")
            lhsT = k_sb[:, kb * 128 : (kb + 1) * 128]
            for qh in range(Q // QH):
                nc.tensor.matmul(
                    s_ps[:, qh * QH : (qh + 1) * QH],
                    lhsT,
                    q_sb[:, qh * QH : (qh + 1) * QH],
                    start=True,
                    stop=True,
                )
            p_kb = p_all[:, kb * Q : (kb + 1) * Q]
            nc.scalar.activation(p_kb, s_ps[:], AF.Exp, scale=SCALE,
                                 bias=fb_sb[:, kb : kb + 1])

        # s_exp is split in a head (first two k-blocks, no DVE work) and a
        # tail: the head of batch b+1 is emitted before den_pv(b), so the
        # ScalarE exp stream never starves at a batch boundary (exp(b+1,0)
        # only needs S(b+1,0), which the PE runs right after S(b)'s tail),
        # while den_pv(b)'s PE/DVE work still fills the PSUM-recycle waits
        # and lands before batch b+1's DVE adds.
        HEAD = 3  # head tiles: 3rd one parks on the PSUM-recycle wait so
        # the exp stream stays dense across the den_pv(b) PE work

        def s_exp_head(b, q_sb, k_sb, fb_sb):
            KB = kb_counts[b]
            p_all = p_pool.tile([128, KBM * Q], ldt, tag="p", name=f"p{b}")
            for kb in range(min(HEAD, KB)):
                s_exp_one(b, kb, q_sb, k_sb, fb_sb, p_all)
            return p_all

        def s_exp_tail(b, q_sb, k_sb, fb_sb, p_all):
            KB = kb_counts[b]
            if KB == 1:
                return None
            pacc = pa_pool.tile([128, Q], ldt, tag="pa", name=f"pa{b}")
            nc.vector.tensor_add(pacc[:], p_all[:, 0:Q], p_all[:, Q : 2 * Q])
            for kb in range(2, KB):
                if kb >= HEAD:
                    s_exp_one(b, kb, q_sb, k_sb, fb_sb, p_all)
                # accumulate P tiles for the denominator as soon as each exp
                # lands; the DVE chain trails the ScalarE stream
                nc.vector.tensor_add(
                    pacc[:], pacc[:], p_all[:, kb * Q : (kb + 1) * Q])
            return pacc

        def den_pv_stage(b, p_all, v_sb, pacc):
            KB = kb_counts[b]
            last = b == BPC - 1
            # O^T[v, q] accumulated over k-blocks, V stationary (kb-outer)
            o_ps = [o_pool.tile([128, QH], f32, tag=f"o{qh}", name=f"o_ps{b}_{qh}")
                    for qh in range(Q // QH)]
            for kb in range(KB):
                for qh in range(Q // QH):
                    nc.tensor.matmul(
                        o_ps[qh][:],
                        v_sb[:, kb * 128 : (kb + 1) * 128],
                        p_all[:, kb * Q + qh * QH : kb * Q + (qh + 1) * QH],
                        start=(kb == 0),
                        stop=(kb == KB - 1),
                    )
            # denominator: one moving pass over the accumulated P, ones
            # stationary (2x512 rows vs KB*2x512 for a mask-matmul pass)
            d_ps = d_pool.tile([1, Q], f32, tag="d", name=f"d_ps{b}")
            dsrc = pacc if pacc is not None else p_all
            for qh in range(Q // QH):
                nc.tensor.matmul(
                    d_ps[:, qh * QH : (qh + 1) * QH],
                    ones_sb[:, 0:1],
                    dsrc[:, qh * QH : (qh + 1) * QH],
                    start=True,
                    stop=True,
                )
            # PSUM can't DMA directly and only ACT/DVE can read PSUM.
            den_sb = e_pool.tile([1, Q], f32, tag="densb", name=f"den_sb{b}")
            o_all = e_pool.tile([128, Q], ldt, tag="oall", name=f"o_all{b}")
            if last:
                # tail ordering: the big O^T halves evac and fly first, the
                # casts split across DVE and Scalar (its exp stream is done),
                # then the small den halves follow on two DGE rings
                nc.vector.tensor_copy(o_all[:, 0:QH], o_ps[0][:])
                nc.sync.dma_start(out=outT[b][:, 0:QH], in_=o_all[:, 0:QH])
                nc.scalar.copy(o_all[:, QH:Q], o_ps[1][:])
                nc.scalar.dma_start(out=outT[b][:, QH:Q], in_=o_all[:, QH:Q])
                nc.vector.tensor_copy(den_sb[:, 0:QH], d_ps[:, 0:QH])
                nc.sync.dma_start(out=den[b][:, 0:QH], in_=den_sb[:, 0:QH])
                nc.scalar.copy(den_sb[:, QH:Q], d_ps[:, QH:Q])
                nc.scalar.dma_start(out=den[b][:, QH:Q], in_=den_sb[:, QH:Q])
            else:
                nc.vector.tensor_copy(den_sb[:], d_ps[:])
                nc.sync.dma_start(out=den[b], in_=den_sb[:])
                # evac with fp16 conversion on DVE: halves the output DMA
                # bytes; the host divides by den in f32 anyway. Single
                # fully-contiguous DMA -> large packets.
                for qh in range(Q // QH):
                    nc.vector.tensor_copy(
                        o_all[:, qh * QH : (qh + 1) * QH], o_ps[qh][:])
                nc.sync.dma_start(out=outT[b], in_=o_all[:])

        # batch-0 loads are emitted first so the k0 issue is the scalar
        # engine's first instruction (its exp work all comes later)
        loads = [load_batch(0)]

        # ones column for the denominator matmul
        ones_sb = c_pool.tile([128, 1], ldt, tag="ones", bufs=1)
        nc.gpsimd.memset(ones_sb[:], 1.0)
        # dummy 1-column exp: hoists the compiler-inserted ACT_TABLE_LOAD
        # (~1.3us) into the batch-0 DMA shadow
        scratch = c_pool.tile([128, 1], ldt, tag="scratch", bufs=1)
        nc.scalar.activation(scratch[:], ones_sb[:], AF.Exp, scale=1.0)

        # HAM pre-warm: dummy matmuls with no data deps run while the batch-0
        # loads are in flight, ramping the PE p-state (a cold PE runs its
        # first ~3us at reduced clock) and covering the DMA latency.
        warm_w = c_pool.tile([128, QH], qk_dt, tag="warmw", bufs=1)
        nc.gpsimd.memset(warm_w[:], 0.0)
        for w in range(N_WARM):
            warm_ps = s_pool.tile([128, QH], f32, tag="s", name=f"warm{w}")
            nc.tensor.matmul(warm_ps[:], warm_w[:, 0:128], warm_w[:],
                             start=True, stop=True)

        # Software pipeline (see s_exp_head comment): per iteration, emit
        # load(b+1), s_exp_tail(b), s_exp_head(b+1), den_pv(b).
        p_alls = [s_exp_head(0, loads[0][0], loads[0][1], loads[0][3])]
        for b in range(BPC):
            if b + 1 < BPC:
                loads.append(load_batch(b + 1))
            q_sb, k_sb, v_sb, fb_sb = loads[b]
            pacc = s_exp_tail(b, q_sb, k_sb, fb_sb, p_alls[b])
            if b + 1 < BPC:
                lq, lk, lv, lfb = loads[b + 1]
                p_alls.append(s_exp_head(b + 1, lq, lk, lfb))
            den_pv_stage(b, p_alls[b], v_sb, pacc)


def _build(kb_counts, sdt):
    key = (tuple(kb_counts), sdt)
    if key in _NC_CACHE:
        return _NC_CACHE[key]
    nc = bacc.Bacc("TRN2", target_bir_lowering=False, debug=False,
                   enable_asserts=False, enable_partition_id=False)
    f32 = mybir.dt.float32
    qk_dt, ldt = _dtypes(sdt)
    qT = nc.dram_tensor("qT", [BPC, D, Q], qk_dt, kind="ExternalInput").ap()
    kts, vts, fbs = [], [], []
    for b in range(BPC):
        KC = kb_counts[b] * 128
        kts.append(nc.dram_tensor(f"k{b}", [D, KC], qk_dt,
                                  kind="ExternalInput").ap())
        vts.append(nc.dram_tensor(f"v{b}", [128, KC], ldt,
                                  kind="ExternalInput").ap())
        fbs.append(nc.dram_tensor(f"fb{b}", [128, kb_counts[b]], f32,
                                  kind="ExternalInput").ap())
    outT = nc.dram_tensor("outT", [BPC, D, Q], ldt, kind="ExternalOutput").ap()
    den = nc.dram_tensor("den", [BPC, 1, Q], f32, kind="ExternalOutput").ap()
    with tile.TileContext(nc) as tc:
        _body(tc, qT, kts, vts, fbs, outT, den, kb_counts, sdt)
    nc.compile()
    _NC_CACHE[key] = nc
    return nc


def _prep(queries, keys, values, valid_length):
    """Returns (in_maps, assign, kb_counts). assign[j, c] = original batch index
    handled by core c slot j."""
    vl = np.asarray(valid_length).astype(np.int64).reshape(B)
    if NO_SPECIALIZE:
        assign = np.arange(B).reshape(N_CORES, BPC).T
        kb_counts = tuple([KB_MAX] * BPC)
    else:
        # sort desc so each slot groups similar lengths across cores, then
        # process slots smallest-KB first: batch 0's k is tiny (arrives
        # fast, exp stream starts early) and later big k's hide behind
        # earlier compute
        order = np.argsort(-vl, kind="stable")
        assign = order.reshape(BPC, N_CORES)[::-1]  # [slot, core]
        kb_counts = tuple(
            max(1, int(np.ceil(vl[assign[j]].max() / 128.0))) for j in range(BPC)
        )

    qk_np = {"fp16": np.float16, "bf16": ml_dtypes.bfloat16,
             "f32r": np.float32, "f32": np.float32}[S_DTYPE]
    ldt_np = np.float16 if S_DTYPE == "fp16" else ml_dtypes.bfloat16
    q = np.asarray(queries, dtype=np.float32)
    k = np.asarray(keys, dtype=np.float32)
    v = np.asarray(values, dtype=np.float32)
    pos = np.arange(K)

    in_maps = []
    for c in range(N_CORES):
        bidx = assign[:, c]
        qTc = np.ascontiguousarray(q[bidx].transpose(0, 2, 1)).astype(qk_np)
        m = {"qT": qTc}
        for j in range(BPC):
            bi = bidx[j]
            KB = kb_counts[j]
            KC = KB * 128
            m[f"k{j}"] = np.ascontiguousarray(
                k[bi, :KC].T).astype(qk_np)  # [D, KC]
            m[f"v{j}"] = np.ascontiguousarray(
                v[bi, :KC].reshape(KB, 128, D).transpose(1, 0, 2).reshape(
                    128, KC)).astype(ldt_np)
            fb = np.where(pos[:KC] < vl[bi], 0.0, MASK_BIAS).astype(np.float32)
            m[f"fb{j}"] = np.ascontiguousarray(
                fb.reshape(KB, 128).T)  # [128, KB]
        in_maps.append(m)
    return in_maps, assign, kb_counts


def kernel(queries, keys, values, valid_length):
    global LAST_RESULTS
    in_maps, assign, kb_counts = _prep(queries, keys, values, valid_length)
    nc = _build(kb_counts, S_DTYPE)
    res = run_bass_kernel_spmd(nc, in_maps, list(range(N_CORES)))
    LAST_RESULTS = res
    out = np.empty((B, Q, D), np.float32)
    for c in range(N_CORES):
        oT = np.asarray(res.results[c]["outT"]).astype(np.float32)  # [BPC,D,Q]
        den = np.asarray(res.results[c]["den"], dtype=np.float32)  # [BPC, 1, Q]
        o = (oT / den).transpose(0, 2, 1)
        for j in range(BPC):
            out[assign[j, c]] = o[j]
    return out


# revision 21
# speedup vs baseline: 1.0525x; 1.0525x over previous
"""Trainium2 Bass/Tile kernel: batched dot-product attention with length masking.

Problem: queries/keys/values [32, 1024, 128] f32, valid_length [32] int64.
  out = softmax(mask(Q K^T / sqrt(128))) @ V

Strategy:
  - Data-parallel: 32 batches sharded 4-per-core across 8 NeuronCores (SPMD,
    identical program, per-core input maps).
  - Host prep per batch (layout only; every tensor is a single fully
    contiguous DMA so descriptors aggregate into large packets):
      qT      [128=D, 1024] f32->fp16  (contraction dim on partitions)
      k{b}    [128=D, KC]   fp16       (K^T trimmed to the live k-blocks)
      v{b}    [128, KB*128] fp16       (V partition-major per k-block)
      fb{b}   [128, KB]     f32        exp-bias: 0 for valid k, -1e4 masked
  - Device per batch (matmul passes stream 512-row moving operands so the
    PE keeps its stationary loaded across 1024 rows):
      S^T[k, q] = (K^T_kb).T @ Q^T           PE
      P^T_kb    = exp(S^T*scale + fb[:,kb])  ScalarE PSUM->SBUF fp16.
                  The per-partition bias is -1e4 on masked k rows, so exp
                  underflows to exactly 0 there: masking costs nothing and
                  no separate mask matmul or V-zeroing is needed.
      pacc      = sum_kb P^T_kb              DVE adds (cheap, off PE)
      den[1,q]  = ones.T @ pacc              PE, only 2x512 rows per batch
                                             (vs KB*2x512 for a full
                                             mask-stationary den pass)
      O^T[v,q]  = sum_kb V_kb @ P^T_kb       PE, V stationary
    The last batch skips pacc and accumulates den over the P tiles directly
    on the PE (KB is smallest there after the sort) so the tail has no
    DVE dependency. Host does out = O^T.T / den in f32.
    No rowmax subtraction needed: scores ~ N(0,1), |S*scale| <~ 6.
  - DMA issues avoid GpSimd entirely: its DGE ring is software-managed and
    costs ~3us in the end-of-kernel drain (sync/scalar/vector rings are HW).
    k0 goes on scalar + q0 on sync so both batch-0 S operands issue in
    parallel at t=0; v's on vector; everything else on sync, with batch b+1
    loads emitted before den_pv(b-1) so output DMAs never delay loads.
  - A dummy 1-column exp at kernel start pulls the ~1.3us ACT_TABLE_LOAD
    into the initial DMA shadow (the compiler inserts it before the first
    Exp on the scalar engine).
  - Length specialization: batches sorted by valid_length desc, assigned
    round-robin so slot j is similar across cores; program compiled per
    kb_counts skips fully-masked k-blocks.
"""

import os

import numpy as np
import ml_dtypes

import concourse.tile as tile
from concourse import bacc, mybir
from concourse.bass_utils import run_bass_kernel_spmd

B, Q, K, D = 32, 1024, 1024, 128
N_CORES = 8
BPC = B // N_CORES  # batches per core
KB_MAX = K // 128
QH = 512
SCALE = float(1.0 / np.sqrt(D))
MASK_BIAS = -10000.0  # exp(s*scale + MASK_BIAS) underflows to exactly 0

S_DTYPE = os.environ.get("ATTN_S_DTYPE", "fp16")  # fp16 | bf16 | f32r | f32
NO_SPECIALIZE = os.environ.get("ATTN_NO_SPECIALIZE", "0") == "1"
N_WARM = int(os.environ.get("ATTN_WARM", "7"))

LAST_RESULTS = None
_NC_CACHE: dict = {}


def _dtypes(sdt):
    """(qk_dt for Q/K/S-matmul, ldt for P/V/ones)."""
    f32 = mybir.dt.float32
    qk = {"fp16": mybir.dt.float16, "bf16": mybir.dt.bfloat16,
          "f32r": mybir.dt.float32r, "f32": f32}[sdt]
    ldt = mybir.dt.float16 if sdt == "fp16" else mybir.dt.bfloat16
    return qk, ldt


def _body(tc, qT, kts, vts, fbs, outT, den, kb_counts, sdt):
    nc = tc.nc
    f32 = mybir.dt.float32
    AF = mybir.ActivationFunctionType
    qk_dt, ldt = _dtypes(sdt)

    with (
        tc.tile_pool(name="qk", bufs=3) as qk_pool,
        tc.tile_pool(name="v", bufs=3) as v_pool,
        tc.tile_pool(name="p", bufs=2) as p_pool,
        tc.tile_pool(name="pa", bufs=2) as pa_pool,
        tc.tile_pool(name="fb", bufs=3) as fb_pool,
        tc.tile_pool(name="eps", bufs=2) as e_pool,
        tc.tile_pool(name="const", bufs=1) as c_pool,
        tc.tile_pool(name="spsum", bufs=2, space="PSUM") as s_pool,
        tc.tile_pool(name="opsum", bufs=1, space="PSUM") as o_pool,
        tc.tile_pool(name="dpsum", bufs=1, space="PSUM") as d_pool,
    ):
        KBM = max(kb_counts)

        def load_batch(b):
            # per-tag tile shapes are constant (max KB) so the pool slot
            # size doesn't depend on allocation order; DMAs/compute slice
            KB = kb_counts[b]
            KC = KB * 128
            q_sb = qk_pool.tile([128, Q], qk_dt, tag="q", name=f"q_sb{b}")
            k_sb = qk_pool.tile([128, KBM * 128], qk_dt, tag="k",
                                name=f"k_sb{b}")
            v_sb = v_pool.tile([128, KBM * 128], ldt, tag="v",
                               name=f"v_sb{b}")
            fb_sb = fb_pool.tile([128, KBM], f32, tag="fb",
                                 name=f"fb_sb{b}")
            # two HW DGE rings (only gpsimd/SP/ACT can issue DMAs; gpsimd's
            # software ring costs ~3us in the final drain, so it issues
            # nothing). Slots are ordered smallest-KB first, so batch 0's k
            # is tiny: k0 rides sync while the full q0 rides scalar, and
            # both batch-0 S operands land ~3.5us (the fixed DGE latency)
            # after kernel entry. k1-k3 keep the scalar ring busy before its
            # exp stream starts.
            if b == 0:
                nc.sync.dma_start(out=k_sb[:, 0:KC], in_=kts[b][:])
                nc.scalar.dma_start(out=q_sb[:], in_=qT[b])
                nc.sync.dma_start(out=fb_sb[:, 0:KB], in_=fbs[b][:])
                nc.scalar.dma_start(out=v_sb[:, 0:KC], in_=vts[b][:])
                return q_sb, k_sb, v_sb, fb_sb
            elif b == 1:
                # k1 is needed early; the scalar ring is still busy with q0.
                # Completion is ~issue-end + 3.5us fixed DGE latency, so
                # queue position directly sets arrival time.
                nc.sync.dma_start(out=k_sb[:, 0:KC], in_=kts[b][:])
                nc.sync.dma_start(out=q_sb[:], in_=qT[b])
            else:
                nc.scalar.dma_start(out=k_sb[:, 0:KC], in_=kts[b][:])
                nc.sync.dma_start(out=q_sb[:], in_=qT[b])
            nc.sync.dma_start(out=fb_sb[:, 0:KB], in_=fbs[b][:])
            nc.sync.dma_start(out=v_sb[:, 0:KC], in_=vts[b][:])
            return q_sb, k_sb, v_sb, fb_sb

        def s_exp_one(b, kb, q_sb, k_sb, fb_sb, p_all):
            s_ps = s_pool.tile([128, Q], f32, tag="s", name=f"s_ps{b}_{kb}")
            lhsT = k_sb[:, kb * 128 : (kb + 1) * 128]
            for qh in range(Q // QH):
                nc.tensor.matmul(
                    s_ps[:, qh * QH : (qh + 1) * QH],
                    lhsT,
                    q_sb[:, qh * QH : (qh + 1) * QH],
                    start=True,
                    stop=True,
                )
            p_kb = p_all[:, kb * Q : (kb + 1) * Q]
            nc.scalar.activation(p_kb, s_ps[:], AF.Exp, scale=SCALE,
                                 bias=fb_sb[:, kb : kb + 1])

        # s_exp is split in a head (first two k-blocks, no DVE work) and a
        # tail: the head of batch b+1 is emitted before den_pv(b), so the
        # ScalarE exp stream never starves at a batch boundary (exp(b+1,0)
        # only needs S(b+1,0), which the PE runs right after S(b)'s tail),
        # while den_pv(b)'s PE/DVE work still fills the PSUM-recycle waits
        # and lands before batch b+1's DVE adds.
        def s_exp_head(b, q_sb, k_sb, fb_sb):
            KB = kb_counts[b]
            p_all = p_pool.tile([128, KBM * Q], ldt, tag="p", name=f"p{b}")
            for kb in range(min(2, KB)):
                s_exp_one(b, kb, q_sb, k_sb, fb_sb, p_all)
            return p_all

        def s_exp_tail(b, q_sb, k_sb, fb_sb, p_all):
            KB = kb_counts[b]
            if KB == 1:
                return None
            pacc = pa_pool.tile([128, Q], ldt, tag="pa", name=f"pa{b}")
            nc.vector.tensor_add(pacc[:], p_all[:, 0:Q], p_all[:, Q : 2 * Q])
            for kb in range(2, KB):
                s_exp_one(b, kb, q_sb, k_sb, fb_sb, p_all)
                # accumulate P tiles for the denominator as soon as each exp
                # lands; the DVE chain trails the ScalarE stream
                nc.vector.tensor_add(
                    pacc[:], pacc[:], p_all[:, kb * Q : (kb + 1) * Q])
            return pacc

        def den_pv_stage(b, p_all, v_sb, pacc):
            KB = kb_counts[b]
            last = b == BPC - 1
            # O^T[v, q] accumulated over k-blocks, V stationary (kb-outer)
            o_ps = [o_pool.tile([128, QH], f32, tag=f"o{qh}", name=f"o_ps{b}_{qh}")
                    for qh in range(Q // QH)]
            for kb in range(KB):
                for qh in range(Q // QH):
                    nc.tensor.matmul(
                        o_ps[qh][:],
                        v_sb[:, kb * 128 : (kb + 1) * 128],
                        p_all[:, kb * Q + qh * QH : kb * Q + (qh + 1) * QH],
                        start=(kb == 0),
                        stop=(kb == KB - 1),
                    )
            # denominator: one moving pass over the accumulated P, ones
            # stationary (2x512 rows vs KB*2x512 for a mask-matmul pass)
            d_ps = d_pool.tile([1, Q], f32, tag="d", name=f"d_ps{b}")
            dsrc = pacc if pacc is not None else p_all
            for qh in range(Q // QH):
                nc.tensor.matmul(
                    d_ps[:, qh * QH : (qh + 1) * QH],
                    ones_sb[:, 0:1],
                    dsrc[:, qh * QH : (qh + 1) * QH],
                    start=True,
                    stop=True,
                )
            # PSUM can't DMA directly and only ACT/DVE can read PSUM.
            den_sb = e_pool.tile([1, Q], f32, tag="densb", name=f"den_sb{b}")
            o_all = e_pool.tile([128, Q], ldt, tag="oall", name=f"o_all{b}")
            if last:
                # tail ordering: the big O^T halves evac and fly first, the
                # casts split across DVE and Scalar (its exp stream is done),
                # then the small den halves follow on two DGE rings
                nc.vector.tensor_copy(o_all[:, 0:QH], o_ps[0][:])
                nc.sync.dma_start(out=outT[b][:, 0:QH], in_=o_all[:, 0:QH])
                nc.scalar.copy(o_all[:, QH:Q], o_ps[1][:])
                nc.scalar.dma_start(out=outT[b][:, QH:Q], in_=o_all[:, QH:Q])
                nc.vector.tensor_copy(den_sb[:, 0:QH], d_ps[:, 0:QH])
                nc.sync.dma_start(out=den[b][:, 0:QH], in_=den_sb[:, 0:QH])
                nc.scalar.copy(den_sb[:, QH:Q], d_ps[:, QH:Q])
                nc.scalar.dma_start(out=den[b][:, QH:Q], in_=den_sb[:, QH:Q])
            else:
                nc.vector.tensor_copy(den_sb[:], d_ps[:])
                nc.sync.dma_start(out=den[b], in_=den_sb[:])
                # evac with fp16 conversion on DVE: halves the output DMA
                # bytes; the host divides by den in f32 anyway. Single
                # fully-contiguous DMA -> large packets.
                for qh in range(Q // QH):
                    nc.vector.tensor_copy(
                        o_all[:, qh * QH : (qh + 1) * QH], o_ps[qh][:])
                nc.sync.dma_start(out=outT[b], in_=o_all[:])

        # batch-0 loads are emitted first so the k0 issue is the scalar
        # engine's first instruction (its exp work all comes later)
        loads = [load_batch(0)]

        # ones column for the denominator matmul
        ones_sb = c_pool.tile([128, 1], ldt, tag="ones", bufs=1)
        nc.gpsimd.memset(ones_sb[:], 1.0)
        # dummy 1-column exp: hoists the compiler-inserted ACT_TABLE_LOAD
        # (~1.3us) into the batch-0 DMA shadow
        scratch = c_pool.tile([128, 1], ldt, tag="scratch", bufs=1)
        nc.scalar.activation(scratch[:], ones_sb[:], AF.Exp, scale=1.0)

        # HAM pre-warm: dummy matmuls with no data deps run while the batch-0
        # loads are in flight, ramping the PE p-state (a cold PE runs its
        # first ~3us at reduced clock) and covering the DMA latency.
        warm_w = c_pool.tile([128, QH], qk_dt, tag="warmw", bufs=1)
        nc.gpsimd.memset(warm_w[:], 0.0)
        for w in range(N_WARM):
            warm_ps = s_pool.tile([128, QH], f32, tag="s", name=f"warm{w}")
            nc.tensor.matmul(warm_ps[:], warm_w[:, 0:128], warm_w[:],
                             start=True, stop=True)

        # Software pipeline (see s_exp_head comment): per iteration, emit
        # load(b+1), s_exp_tail(b), s_exp_head(b+1), den_pv(b).
        p_alls = [s_exp_head(0, loads[0][0], loads[0][1], loads[0][3])]
        for b in range(BPC):
            if b + 1 < BPC:
                loads.append(load_batch(b + 1))
            q_sb, k_sb, v_sb, fb_sb = loads[b]
            pacc = s_exp_tail(b, q_sb, k_sb, fb_sb, p_alls[b])
            if b + 1 < BPC:
                lq, lk, lv, lfb = loads[b + 1]
                p_alls.append(s_exp_head(b + 1, lq, lk, lfb))
            den_pv_stage(b, p_alls[b], v_sb, pacc)


def _build(kb_counts, sdt):
    key = (tuple(kb_counts), sdt)
    if key in _NC_CACHE:
        return _NC_CACHE[key]
    nc = bacc.Bacc("TRN2", target_bir_lowering=False, debug=False,
                   enable_asserts=False, enable_partition_id=False)
    f32 = mybir.dt.float32
    qk_dt, ldt = _dtypes(sdt)
    qT = nc.dram_tensor("qT", [BPC, D, Q], qk_dt, kind="ExternalInput").ap()
    kts, vts, fbs = [], [], []
    for b in range(BPC):
        KC = kb_counts[b] * 128
        kts.append(nc.dram_tensor(f"k{b}", [D, KC], qk_dt,
                                  kind="ExternalInput").ap())
        vts.append(nc.dram_tensor(f"v{b}", [128, KC], ldt,
                                  kind="ExternalInput").ap())
        fbs.append(nc.dram_tensor(f"fb{b}", [128, kb_counts[b]], f32,
                                  kind="ExternalInput").ap())
    outT = nc.dram_tensor("outT", [BPC, D, Q], ldt, kind="ExternalOutput").ap()
    den = nc.dram_tensor("den", [BPC, 1, Q], f32, kind="ExternalOutput").ap()
    with tile.TileContext(nc) as tc:
        _body(tc, qT, kts, vts, fbs, outT, den, kb_counts, sdt)
    nc.compile()
    _NC_CACHE[key] = nc
    return nc


def _prep(queries, keys, values, valid_length):
    """Returns (in_maps, assign, kb_counts). assign[j, c] = original batch index
    handled by core c slot j."""
    vl = np.asarray(valid_length).astype(np.int64).reshape(B)
    if NO_SPECIALIZE:
        assign = np.arange(B).reshape(N_CORES, BPC).T
        kb_counts = tuple([KB_MAX] * BPC)
    else:
        # sort desc so each slot groups similar lengths across cores, then
        # process slots smallest-KB first: batch 0's k is tiny (arrives
        # fast, exp stream starts early) and later big k's hide behind
        # earlier compute
        order = np.argsort(-vl, kind="stable")
        assign = order.reshape(BPC, N_CORES)[::-1]  # [slot, core]
        kb_counts = tuple(
            max(1, int(np.ceil(vl[assign[j]].max() / 128.0))) for j in range(BPC)
        )

    qk_np = {"fp16": np.float16, "bf16": ml_dtypes.bfloat16,
             "f32r": np.float32, "f32": np.float32}[S_DTYPE]
    ldt_np = np.float16 if S_DTYPE == "fp16" else ml_dtypes.bfloat16
    q = np.asarray(queries, dtype=np.float32)
    k = np.asarray(keys, dtype=np.float32)
    v = np.asarray(values, dtype=np.float32)
    pos = np.arange(K)

    in_maps = []
    for c in range(N_CORES):
        bidx = assign[:, c]
        qTc = np.ascontiguousarray(q[bidx].transpose(0, 2, 1)).astype(qk_np)
        m = {"qT": qTc}
        for j in range(BPC):
            bi = bidx[j]
            KB = kb_counts[j]
            KC = KB * 128
            m[f"k{j}"] = np.ascontiguousarray(
                k[bi, :KC].T).astype(qk_np)  # [D, KC]
            m[f"v{j}"] = np.ascontiguousarray(
                v[bi, :KC].reshape(KB, 128, D).transpose(1, 0, 2).reshape(
                    128, KC)).astype(ldt_np)
            fb = np.where(pos[:KC] < vl[bi], 0.0, MASK_BIAS).astype(np.float32)
            m[f"fb{j}"] = np.ascontiguousarray(
                fb.reshape(KB, 128).T)  # [128, KB]
        in_maps.append(m)
    return in_maps, assign, kb_counts


def kernel(queries, keys, values, valid_length):
    global LAST_RESULTS
    in_maps, assign, kb_counts = _prep(queries, keys, values, valid_length)
    nc = _build(kb_counts, S_DTYPE)
    res = run_bass_kernel_spmd(nc, in_maps, list(range(N_CORES)))
    LAST_RESULTS = res
    out = np.empty((B, Q, D), np.float32)
    for c in range(N_CORES):
        oT = np.asarray(res.results[c]["outT"]).astype(np.float32)  # [BPC,D,Q]
        den = np.asarray(res.results[c]["den"], dtype=np.float32)  # [BPC, 1, Q]
        o = (oT / den).transpose(0, 2, 1)
        for j in range(BPC):
            out[assign[j, c]] = o[j]
    return out


# revision 22
# speedup vs baseline: 1.1111x; 1.0557x over previous
"""Trainium2 Bass/Tile kernel: batched dot-product attention with length masking.

Problem: queries/keys/values [32, 1024, 128] f32, valid_length [32] int64.
  out = softmax(mask(Q K^T / sqrt(128))) @ V

Strategy:
  - Data-parallel: 32 batches sharded 4-per-core across 8 NeuronCores (SPMD,
    identical program, per-core input maps).
  - Host prep per batch (layout only; every tensor is a single fully
    contiguous DMA so descriptors aggregate into large packets):
      qT      [128=D, 1024] f32->fp16  (contraction dim on partitions)
      k{b}    [128=D, KC]   fp16       (K^T trimmed to the live k-blocks)
      v{b}    [128, KB*128] fp16       (V partition-major per k-block)
      fb{b}   [128, KB]     f32        exp-bias: 0 for valid k, -1e4 masked
  - Device per batch (matmul passes stream 512-row moving operands so the
    PE keeps its stationary loaded across 1024 rows):
      S^T[k, q] = (K^T_kb).T @ Q^T           PE
      P^T_kb    = exp(S^T*scale + fb[:,kb])  ScalarE PSUM->SBUF fp16.
                  The per-partition bias is -1e4 on masked k rows, so exp
                  underflows to exactly 0 there: masking costs nothing and
                  no separate mask matmul or V-zeroing is needed.
      pacc      = sum_kb P^T_kb              DVE adds (cheap, off PE)
      den[1,q]  = ones.T @ pacc              PE, only 2x512 rows per batch
                                             (vs KB*2x512 for a full
                                             mask-stationary den pass)
      O^T[v,q]  = sum_kb V_kb @ P^T_kb       PE, V stationary
    The last batch skips pacc and accumulates den over the P tiles directly
    on the PE (KB is smallest there after the sort) so the tail has no
    DVE dependency. Host does out = O^T.T / den in f32.
    No rowmax subtraction needed: scores ~ N(0,1), |S*scale| <~ 6.
  - DMA issues avoid GpSimd entirely: its DGE ring is software-managed and
    costs ~3us in the end-of-kernel drain (sync/scalar/vector rings are HW).
    k0 goes on scalar + q0 on sync so both batch-0 S operands issue in
    parallel at t=0; v's on vector; everything else on sync, with batch b+1
    loads emitted before den_pv(b-1) so output DMAs never delay loads.
  - A dummy 1-column exp at kernel start pulls the ~1.3us ACT_TABLE_LOAD
    into the initial DMA shadow (the compiler inserts it before the first
    Exp on the scalar engine).
  - Length specialization: batches sorted by valid_length desc, assigned
    round-robin so slot j is similar across cores; program compiled per
    kb_counts skips fully-masked k-blocks.
"""

import os

import numpy as np
import ml_dtypes

import concourse.tile as tile
from concourse import bacc, mybir
from concourse.bass_utils import run_bass_kernel_spmd

B, Q, K, D = 32, 1024, 1024, 128
N_CORES = 8
BPC = B // N_CORES  # batches per core
KB_MAX = K // 128
QH = 512
SCALE = float(1.0 / np.sqrt(D))
MASK_BIAS = -10000.0  # exp(s*scale + MASK_BIAS) underflows to exactly 0

S_DTYPE = os.environ.get("ATTN_S_DTYPE", "fp16")  # fp16 | bf16 | f32r | f32
NO_SPECIALIZE = os.environ.get("ATTN_NO_SPECIALIZE", "0") == "1"
N_WARM = int(os.environ.get("ATTN_WARM", "7"))

LAST_RESULTS = None
_NC_CACHE: dict = {}


def _dtypes(sdt):
    """(qk_dt for Q/K/S-matmul, ldt for P/V/ones)."""
    f32 = mybir.dt.float32
    qk = {"fp16": mybir.dt.float16, "bf16": mybir.dt.bfloat16,
          "f32r": mybir.dt.float32r, "f32": f32}[sdt]
    ldt = mybir.dt.float16 if sdt == "fp16" else mybir.dt.bfloat16
    return qk, ldt


def _body(tc, qT, kts, vts, fbs, outT, den, kb_counts, sdt):
    nc = tc.nc
    f32 = mybir.dt.float32
    AF = mybir.ActivationFunctionType
    qk_dt, ldt = _dtypes(sdt)

    with (
        tc.tile_pool(name="qk", bufs=3) as qk_pool,
        tc.tile_pool(name="v", bufs=3) as v_pool,
        tc.tile_pool(name="p", bufs=2) as p_pool,
        tc.tile_pool(name="pa", bufs=2) as pa_pool,
        tc.tile_pool(name="fb", bufs=3) as fb_pool,
        tc.tile_pool(name="eps", bufs=2) as e_pool,
        tc.tile_pool(name="const", bufs=1) as c_pool,
        tc.tile_pool(name="spsum", bufs=2, space="PSUM") as s_pool,
        tc.tile_pool(name="opsum", bufs=1, space="PSUM") as o_pool,
        tc.tile_pool(name="dpsum", bufs=1, space="PSUM") as d_pool,
    ):
        KBM = max(kb_counts)

        def load_batch(b):
            # per-tag tile shapes are constant (max KB) so the pool slot
            # size doesn't depend on allocation order; DMAs/compute slice
            KB = kb_counts[b]
            KC = KB * 128
            q_sb = qk_pool.tile([128, Q], qk_dt, tag="q", name=f"q_sb{b}")
            k_sb = qk_pool.tile([128, KBM * 128], qk_dt, tag="k",
                                name=f"k_sb{b}")
            v_sb = v_pool.tile([128, KBM * 128], ldt, tag="v",
                               name=f"v_sb{b}")
            fb_sb = fb_pool.tile([128, KBM], f32, tag="fb",
                                 name=f"fb_sb{b}")
            # two HW DGE rings (only gpsimd/SP/ACT can issue DMAs; gpsimd's
            # software ring costs ~3us in the final drain, so it issues
            # nothing). Slots are ordered smallest-KB first, so batch 0's k
            # is tiny: k0 rides sync while the full q0 rides scalar, and
            # both batch-0 S operands land ~3.5us (the fixed DGE latency)
            # after kernel entry. k1-k3 keep the scalar ring busy before its
            # exp stream starts.
            if b == 0:
                nc.sync.dma_start(out=k_sb[:, 0:KC], in_=kts[b][:])
                nc.scalar.dma_start(out=q_sb[:], in_=qT[b])
                nc.sync.dma_start(out=fb_sb[:, 0:KB], in_=fbs[b][:])
                nc.scalar.dma_start(out=v_sb[:, 0:KC], in_=vts[b][:])
                return q_sb, k_sb, v_sb, fb_sb
            elif b == 1:
                # k1 is needed early; the scalar ring is still busy with q0.
                # Completion is ~issue-end + 3.5us fixed DGE latency, so
                # queue position directly sets arrival time.
                nc.sync.dma_start(out=k_sb[:, 0:KC], in_=kts[b][:])
                nc.sync.dma_start(out=q_sb[:], in_=qT[b])
            else:
                nc.scalar.dma_start(out=k_sb[:, 0:KC], in_=kts[b][:])
                nc.sync.dma_start(out=q_sb[:], in_=qT[b])
            nc.sync.dma_start(out=fb_sb[:, 0:KB], in_=fbs[b][:])
            nc.sync.dma_start(out=v_sb[:, 0:KC], in_=vts[b][:])
            return q_sb, k_sb, v_sb, fb_sb

        def s_exp_one(b, kb, q_sb, k_sb, fb_sb, p_all):
            s_ps = s_pool.tile([128, Q], f32, tag="s", name=f"s_ps{b}_{kb}")
            lhsT = k_sb[:, kb * 128 : (kb + 1) * 128]
            for qh in range(Q // QH):
                nc.tensor.matmul(
                    s_ps[:, qh * QH : (qh + 1) * QH],
                    lhsT,
                    q_sb[:, qh * QH : (qh + 1) * QH],
                    start=True,
                    stop=True,
                )
            p_kb = p_all[:, kb * Q : (kb + 1) * Q]
            nc.scalar.activation(p_kb, s_ps[:], AF.Exp, scale=SCALE,
                                 bias=fb_sb[:, kb : kb + 1])

        # s_exp is split in a head (first two k-blocks, no DVE work) and a
        # tail: the head of batch b+1 is emitted before den_pv(b), so the
        # ScalarE exp stream never starves at a batch boundary (exp(b+1,0)
        # only needs S(b+1,0), which the PE runs right after S(b)'s tail),
        # while den_pv(b)'s PE/DVE work still fills the PSUM-recycle waits
        # and lands before batch b+1's DVE adds.
        def s_exp_head(b, q_sb, k_sb, fb_sb):
            KB = kb_counts[b]
            p_all = p_pool.tile([128, KBM * Q], ldt, tag="p", name=f"p{b}")
            for kb in range(min(2, KB)):
                s_exp_one(b, kb, q_sb, k_sb, fb_sb, p_all)
            return p_all

        def s_exp_tail(b, q_sb, k_sb, fb_sb, p_all, start_kb):
            KB = kb_counts[b]
            if KB == 1:
                return None
            pacc = pa_pool.tile([128, Q], ldt, tag="pa", name=f"pa{b}")
            nc.vector.tensor_add(pacc[:], p_all[:, 0:Q], p_all[:, Q : 2 * Q])
            for kb in range(2, KB):
                if kb >= start_kb:
                    s_exp_one(b, kb, q_sb, k_sb, fb_sb, p_all)
                # accumulate P tiles for the denominator as soon as each exp
                # lands; the DVE chain trails the ScalarE stream
                nc.vector.tensor_add(
                    pacc[:], pacc[:], p_all[:, kb * Q : (kb + 1) * Q])
            return pacc

        def den_pv_stage(b, p_all, v_sb, pacc, nxt=None):
            KB = kb_counts[b]
            last = b == BPC - 1
            # O^T[v, q] accumulated over k-blocks, V stationary (kb-outer)
            o_ps = [o_pool.tile([128, QH], f32, tag=f"o{qh}", name=f"o_ps{b}_{qh}")
                    for qh in range(Q // QH)]
            for kb in range(KB):
                for qh in range(Q // QH):
                    nc.tensor.matmul(
                        o_ps[qh][:],
                        v_sb[:, kb * 128 : (kb + 1) * 128],
                        p_all[:, kb * Q + qh * QH : kb * Q + (qh + 1) * QH],
                        start=(kb == 0),
                        stop=(kb == KB - 1),
                    )
            # next batch's 3rd S tile + exp go here: after PV (so nothing
            # parks the in-order PE queue on its PSUM-recycle wait) but
            # before den, keeping the ScalarE stream dense across the
            # batch boundary
            if nxt is not None:
                nb, nq, nk, nfb, np_all = nxt
                if kb_counts[nb] > 2:
                    s_exp_one(nb, 2, nq, nk, nfb, np_all)
            # denominator: one moving pass over the accumulated P, ones
            # stationary (2x512 rows vs KB*2x512 for a mask-matmul pass)
            d_ps = d_pool.tile([1, Q], f32, tag="d", name=f"d_ps{b}")
            dsrc = pacc if pacc is not None else p_all
            for qh in range(Q // QH):
                nc.tensor.matmul(
                    d_ps[:, qh * QH : (qh + 1) * QH],
                    ones_sb[:, 0:1],
                    dsrc[:, qh * QH : (qh + 1) * QH],
                    start=True,
                    stop=True,
                )
            # PSUM can't DMA directly and only ACT/DVE can read PSUM.
            den_sb = e_pool.tile([1, Q], f32, tag="densb", name=f"den_sb{b}")
            o_all = e_pool.tile([128, Q], ldt, tag="oall", name=f"o_all{b}")
            if last:
                # tail ordering: the big O^T halves evac and fly first, the
                # casts split across DVE and Scalar (its exp stream is done),
                # then the small den halves follow on two DGE rings
                nc.vector.tensor_copy(o_all[:, 0:QH], o_ps[0][:])
                nc.sync.dma_start(out=outT[b][:, 0:QH], in_=o_all[:, 0:QH])
                nc.scalar.copy(o_all[:, QH:Q], o_ps[1][:])
                nc.scalar.dma_start(out=outT[b][:, QH:Q], in_=o_all[:, QH:Q])
                nc.vector.tensor_copy(den_sb[:, 0:QH], d_ps[:, 0:QH])
                nc.sync.dma_start(out=den[b][:, 0:QH], in_=den_sb[:, 0:QH])
                nc.scalar.copy(den_sb[:, QH:Q], d_ps[:, QH:Q])
                nc.scalar.dma_start(out=den[b][:, QH:Q], in_=den_sb[:, QH:Q])
            else:
                nc.vector.tensor_copy(den_sb[:], d_ps[:])
                nc.sync.dma_start(out=den[b], in_=den_sb[:])
                # evac with fp16 conversion on DVE: halves the output DMA
                # bytes; the host divides by den in f32 anyway. Single
                # fully-contiguous DMA -> large packets.
                for qh in range(Q // QH):
                    nc.vector.tensor_copy(
                        o_all[:, qh * QH : (qh + 1) * QH], o_ps[qh][:])
                nc.sync.dma_start(out=outT[b], in_=o_all[:])

        # batch-0 loads are emitted first so the k0 issue is the scalar
        # engine's first instruction (its exp work all comes later)
        loads = [load_batch(0)]

        # ones column for the denominator matmul
        ones_sb = c_pool.tile([128, 1], ldt, tag="ones", bufs=1)
        nc.gpsimd.memset(ones_sb[:], 1.0)
        # dummy 1-column exp: hoists the compiler-inserted ACT_TABLE_LOAD
        # (~1.3us) into the batch-0 DMA shadow
        scratch = c_pool.tile([128, 1], ldt, tag="scratch", bufs=1)
        nc.scalar.activation(scratch[:], ones_sb[:], AF.Exp, scale=1.0)

        # HAM pre-warm: dummy matmuls with no data deps run while the batch-0
        # loads are in flight, ramping the PE p-state (a cold PE runs its
        # first ~3us at reduced clock) and covering the DMA latency.
        warm_w = c_pool.tile([128, QH], qk_dt, tag="warmw", bufs=1)
        nc.gpsimd.memset(warm_w[:], 0.0)
        for w in range(N_WARM):
            warm_ps = s_pool.tile([128, QH], f32, tag="s", name=f"warm{w}")
            nc.tensor.matmul(warm_ps[:], warm_w[:, 0:128], warm_w[:],
                             start=True, stop=True)

        # Software pipeline (see s_exp_head comment): per iteration, emit
        # load(b+1), s_exp_tail(b), s_exp_head(b+1), den_pv(b).
        p_alls = [s_exp_head(0, loads[0][0], loads[0][1], loads[0][3])]
        for b in range(BPC):
            if b + 1 < BPC:
                loads.append(load_batch(b + 1))
            q_sb, k_sb, v_sb, fb_sb = loads[b]
            # batch 0's kb=2 tile has no preceding den_pv to ride in
            pacc = s_exp_tail(b, q_sb, k_sb, fb_sb, p_alls[b],
                              start_kb=2 if b == 0 else 3)
            nxt = None
            if b + 1 < BPC:
                lq, lk, lv, lfb = loads[b + 1]
                p_alls.append(s_exp_head(b + 1, lq, lk, lfb))
                nxt = (b + 1, lq, lk, lfb, p_alls[b + 1])
            den_pv_stage(b, p_alls[b], v_sb, pacc, nxt)


def _build(kb_counts, sdt):
    key = (tuple(kb_counts), sdt)
    if key in _NC_CACHE:
        return _NC_CACHE[key]
    nc = bacc.Bacc("TRN2", target_bir_lowering=False, debug=False,
                   enable_asserts=False, enable_partition_id=False)
    f32 = mybir.dt.float32
    qk_dt, ldt = _dtypes(sdt)
    qT = nc.dram_tensor("qT", [BPC, D, Q], qk_dt, kind="ExternalInput").ap()
    kts, vts, fbs = [], [], []
    for b in range(BPC):
        KC = kb_counts[b] * 128
        kts.append(nc.dram_tensor(f"k{b}", [D, KC], qk_dt,
                                  kind="ExternalInput").ap())
        vts.append(nc.dram_tensor(f"v{b}", [128, KC], ldt,
                                  kind="ExternalInput").ap())
        fbs.append(nc.dram_tensor(f"fb{b}", [128, kb_counts[b]], f32,
                                  kind="ExternalInput").ap())
    outT = nc.dram_tensor("outT", [BPC, D, Q], ldt, kind="ExternalOutput").ap()
    den = nc.dram_tensor("den", [BPC, 1, Q], f32, kind="ExternalOutput").ap()
    with tile.TileContext(nc) as tc:
        _body(tc, qT, kts, vts, fbs, outT, den, kb_counts, sdt)
    nc.compile()
    _NC_CACHE[key] = nc
    return nc


def _prep(queries, keys, values, valid_length):
    """Returns (in_maps, assign, kb_counts). assign[j, c] = original batch index
    handled by core c slot j."""
    vl = np.asarray(valid_length).astype(np.int64).reshape(B)
    if NO_SPECIALIZE:
        assign = np.arange(B).reshape(N_CORES, BPC).T
        kb_counts = tuple([KB_MAX] * BPC)
    else:
        # sort desc so each slot groups similar lengths across cores, then
        # process slots smallest-KB first: batch 0's k is tiny (arrives
        # fast, exp stream starts early) and later big k's hide behind
        # earlier compute
        order = np.argsort(-vl, kind="stable")
        assign = order.reshape(BPC, N_CORES)[::-1]  # [slot, core]
        kb_counts = tuple(
            max(1, int(np.ceil(vl[assign[j]].max() / 128.0))) for j in range(BPC)
        )

    qk_np = {"fp16": np.float16, "bf16": ml_dtypes.bfloat16,
             "f32r": np.float32, "f32": np.float32}[S_DTYPE]
    ldt_np = np.float16 if S_DTYPE == "fp16" else ml_dtypes.bfloat16
    q = np.asarray(queries, dtype=np.float32)
    k = np.asarray(keys, dtype=np.float32)
    v = np.asarray(values, dtype=np.float32)
    pos = np.arange(K)

    in_maps = []
    for c in range(N_CORES):
        bidx = assign[:, c]
        qTc = np.ascontiguousarray(q[bidx].transpose(0, 2, 1)).astype(qk_np)
        m = {"qT": qTc}
        for j in range(BPC):
            bi = bidx[j]
            KB = kb_counts[j]
            KC = KB * 128
            m[f"k{j}"] = np.ascontiguousarray(
                k[bi, :KC].T).astype(qk_np)  # [D, KC]
            m[f"v{j}"] = np.ascontiguousarray(
                v[bi, :KC].reshape(KB, 128, D).transpose(1, 0, 2).reshape(
                    128, KC)).astype(ldt_np)
            fb = np.where(pos[:KC] < vl[bi], 0.0, MASK_BIAS).astype(np.float32)
            m[f"fb{j}"] = np.ascontiguousarray(
                fb.reshape(KB, 128).T)  # [128, KB]
        in_maps.append(m)
    return in_maps, assign, kb_counts


def kernel(queries, keys, values, valid_length):
    global LAST_RESULTS
    in_maps, assign, kb_counts = _prep(queries, keys, values, valid_length)
    nc = _build(kb_counts, S_DTYPE)
    res = run_bass_kernel_spmd(nc, in_maps, list(range(N_CORES)))
    LAST_RESULTS = res
    out = np.empty((B, Q, D), np.float32)
    for c in range(N_CORES):
        oT = np.asarray(res.results[c]["outT"]).astype(np.float32)  # [BPC,D,Q]
        den = np.asarray(res.results[c]["den"], dtype=np.float32)  # [BPC, 1, Q]
        o = (oT / den).transpose(0, 2, 1)
        for j in range(BPC):
            out[assign[j, c]] = o[j]
    return out
